# revision 1
# baseline (speedup 1.0000x reference)
"""Trainium2 Bass kernel for submanifold sparse conv net (gnn_message_passing).

Network: mask = (x != 0); y = BN(x) masked; y1 = relu(subm_conv3x3(y, w1) + b1);
y2 = relu(subm_conv3x3(y1, w2) + b2); out = NCHW(y2).  B,H,W = 4,512,512, C: 1->32->64.

Sharding: H split into 8 slabs of 64 rows (one per NeuronCore), 2-row halo.

Per-core design (all per batch, plane layout = channels on partitions,
pixel index p = row*516 + col on the free dim, 2 zero-pad cols each side):

- conv1 is computed with a single matmul per y1-row: K=30 rhs rows are
  (plane in {x, mask}) x (dh in -1..1) x (dw in -2..2) shifted copies of the
  input plane, materialized for free by a single DMA with a 3-level
  partition access pattern over the padded DRAM slab.  M=96 = 3 "dw groups"
  x 32 channels: group g holds y1 evaluated at column+g-1, so the 3x3 conv2
  later needs only free-dim (row) shifts -> 3 matmuls of K<=98, no im2col
  copies on chip.
- Masking (submanifold restrict) is folded into the matmuls: a +LARGE*mask
  row with bias -LARGE makes relu zero out inactive sites exactly.
- BN is folded into w1 host-side (s*x + t with t*conv(mask,w1) via the mask
  plane rows).  conv2's bias b2 and -LARGE ride on a constant ones row.
- bf16 matmul operands, fp32 PSUM accumulate; relu on ACT (conv1) and DVE
  (conv2) to split elementwise load across engines.
"""

import sys

if "/opt/trn_rl_repo" not in sys.path:
    sys.path.insert(0, "/opt/trn_rl_repo")

import numpy as np
import ml_dtypes

BF16 = ml_dtypes.bfloat16

B, H, W = 4, 512, 512
NCORES = 8
ROWS = H // NCORES          # 64 output rows per core
SLAB = ROWS + 4             # 68 input rows incl. 2-row halo each side
WP = W + 4                  # 516 padded cols
PLANE = SLAB * WP           # 35088
BATCH_ELEMS = 7 * PLANE + 4  # 6 (plane,dh) shifted copies + ones + slack
LARGE = 256.0
EPS = 1e-5
CHUNK = 32                  # output rows per inner tile
LROWS = CHUNK + 2           # y1 rows per chunk (halo 1)
LFREE = LROWS * WP          # 9288 free elems per chunk tile

_cached = {}


def _build_nc():
    import concourse.bass as bass
    import concourse.mybir as mybir
    from concourse import bacc, tile

    f32 = mybir.dt.float32
    bf16 = mybir.dt.bfloat16
    AP = bass.AP

    nc = bacc.Bacc("TRN2", target_bir_lowering=False, debug=False,
                   num_devices=NCORES)
    xm = nc.declare_dram_parameter("xm", [B * BATCH_ELEMS], bf16, isOutput=False)
    wts = nc.declare_dram_parameter("wts", [98 * 288], bf16, isOutput=False)
    bias1d = nc.declare_dram_parameter("bias1", [96], f32, isOutput=False)
    out = nc.declare_dram_parameter("out", [B * 64 * ROWS * W], f32, isOutput=True)

    with tile.TileContext(nc) as tc:
        with (
            tc.tile_pool(name="const", bufs=1) as cpool,
            tc.tile_pool(name="rhs1", bufs=2) as rpool,
            tc.tile_pool(name="y1", bufs=2) as ypool,
            tc.tile_pool(name="stage", bufs=6) as spool,
            tc.tile_pool(name="ps1", bufs=2, space="PSUM") as p1pool,
            tc.tile_pool(name="ps2", bufs=2, space="PSUM") as p2pool,
        ):
            wcat = cpool.tile([98, 288], bf16, tag="wcat")
            bias1 = cpool.tile([96, 1], f32, tag="bias1")
            nc.sync.dma_start(out=wcat[:, :], in_=AP(wts, 0, [[288, 98], [1, 288]]))
            nc.sync.dma_start(out=bias1[:, :], in_=AP(bias1d, 0, [[1, 96], [1, 1]]))
            w1t = wcat[0:30, 0:96]
            w2a = wcat[0:96, 96:160]
            w2b = wcat[0:98, 160:224]
            w2c = wcat[0:96, 224:288]
            # warm engines against the const DMA lanes so first real ops
            # don't exceed the per-instruction sync-wait limit
            scratch = cpool.tile([96, 1], f32, tag="scratch")
            nc.scalar.activation(scratch[:, :], bias1[:, 0:1],
                                 mybir.ActivationFunctionType.Relu,
                                 bias=bias1[:, 0:1])

            for b in range(B):
                boff = b * BATCH_ELEMS
                for k4 in range(ROWS // CHUNK):
                    r0 = k4 * CHUNK
                    # --- load shifted input planes: rhs1[kp, rrl*516+pc] =
                    #     P[plane, (r0-1+rrl)+2+dh, pc+dw]
                    rhs1 = rpool.tile([30, LFREE], bf16, tag="rhs1")
                    nc.scalar.dma_start(
                        out=rhs1[:, :],
                        in_=AP(xm, boff + 2 + (r0 + 1) * WP - 2,
                               [[PLANE, 6], [1, 5], [1, LFREE]]),
                    )
                    # y1 stored COMPACT: 512-wide rows (boundary zeros come
                    # from the masked conv1 groups), so conv2 row shifts are
                    # +-512 in the free dim.
                    y1 = ypool.tile([98, LROWS * W], bf16, tag="y1")
                    # mask (center-aligned) + ones planes -> partitions 96,97
                    # (compact: per-row 512 slices of the padded plane)
                    nc.scalar.dma_start(
                        out=y1[96:98, :],
                        in_=AP(xm, boff + 2 + 4 * PLANE + (r0 + 1) * WP + 2,
                               [[2 * PLANE, 2], [WP, LROWS], [1, W]]),
                    )
                    # --- conv1 and conv2 interleaved: conv1 pairs lead by
                    # two so ACT's relu pipeline stays ahead of conv2's reads
                    def conv1_pair(rp):
                        ps1 = p1pool.tile([96, 1024], f32, tag="ps1")
                        for half in range(2):
                            rrl = 2 * rp + half
                            nc.tensor.matmul(
                                ps1[:, 512 * half:512 * half + 512], lhsT=w1t,
                                rhs=rhs1[:, rrl * WP + 2: rrl * WP + 514],
                                start=True, stop=True,
                            )
                        nc.scalar.activation(
                            y1[0:96, 2 * rp * W: 2 * rp * W + 1024], ps1[:, :],
                            mybir.ActivationFunctionType.Relu,
                            bias=bias1[:, 0:1],
                        )

                    conv1_pair(0)
                    conv1_pair(1)
                    for rp in range(CHUNK // 2):
                        if rp + 2 < LROWS // 2 + (LROWS % 2):
                            if rp + 2 < (LROWS + 1) // 2:
                                conv1_pair(rp + 2)
                        ps2 = p2pool.tile([64, 1024], f32, tag="ps2")
                        for wsel, wt, kk in ((0, w2a, 96), (1, w2b, 98), (2, w2c, 96)):
                            for half in range(2):
                                rrl = 2 * rp + half + 1
                                src = (rrl + wsel - 1) * W
                                nc.tensor.matmul(
                                    ps2[:, 512 * half:512 * half + 512], lhsT=wt,
                                    rhs=y1[0:kk, src: src + 512],
                                    start=(wsel == 0), stop=(wsel == 2),
                                )
                        stage = spool.tile([64, 1024], f32, tag="stage")
                        nc.vector.tensor_scalar_max(stage[:, :], ps2[:, :], 0.0)
                        gr = r0 + 2 * rp
                        nc.sync.dma_start(
                            out=AP(out, (b * 64 * ROWS + gr) * W,
                                   [[ROWS * W, 64], [W, 2], [1, W]]),
                            in_=stage[:, :],
                        )
    nc.finalize()
    return nc


def _prep_consts(bn_gamma, bn_beta, bn_mean, bn_var, w1, b1, w2, b2):
    s = float(bn_gamma[0] / np.sqrt(bn_var[0] + EPS))
    t = float(bn_beta[0] - bn_mean[0] * s)
    w1 = np.asarray(w1, np.float32)  # [3,3,1,32] (kh, kw, ci, co)
    w2 = np.asarray(w2, np.float32)  # [3,3,32,64]
    W1t = np.zeros((30, 96), np.float32)
    for plane in range(2):
        for dh in (-1, 0, 1):
            for dw in (-2, -1, 0, 1, 2):
                kp = (plane * 3 + (dh + 1)) * 5 + (dw + 2)
                for g in range(3):
                    dwp = dw - (g - 1)
                    col = slice(g * 32, g * 32 + 32)
                    if -1 <= dwp <= 1:
                        coef = s if plane == 0 else t
                        W1t[kp, col] = coef * w1[dh + 1, dwp + 1, 0, :]
                    if plane == 1 and dh == 0 and dw == (g - 1):
                        W1t[kp, col] += LARGE
    bias1 = np.tile(np.asarray(b1, np.float32), 3) - LARGE  # [96]
    W2a = np.zeros((96, 64), np.float32)
    W2b = np.zeros((98, 64), np.float32)
    W2c = np.zeros((96, 64), np.float32)
    for g in range(3):
        for ci in range(32):
            r = g * 32 + ci
            W2a[r] = w2[0, g, ci, :]
            W2b[r] = w2[1, g, ci, :]
            W2c[r] = w2[2, g, ci, :]
    W2b[96] = LARGE
    W2b[97] = np.asarray(b2, np.float32) - LARGE
    wcat = np.zeros((98, 288), np.float32)
    wcat[0:30, 0:96] = W1t
    wcat[0:96, 96:160] = W2a
    wcat[0:98, 160:224] = W2b
    wcat[0:96, 224:288] = W2c
    return wcat.ravel().astype(BF16), bias1.astype(np.float32)


def _prep_xm(x):
    """Per-core padded planes. x: [B,H,W,1] f32 -> list of 8 flat bf16 arrays."""
    x = np.asarray(x, np.float32)[..., 0]        # [B,H,W]
    mask = (x != 0.0).astype(np.float32)
    xp = np.zeros((B, H + 4, WP), np.float32)
    mp = np.zeros((B, H + 4, WP), np.float32)
    xp[:, 2:H + 2, 2:W + 2] = x
    mp[:, 2:H + 2, 2:W + 2] = mask
    # Q[k] = plane shifted by dh rows: k = plane*3 + (dh+1);  Q[6] = ones
    xpp = np.zeros((B, H + 6, WP), np.float32)
    mpp = np.zeros((B, H + 6, WP), np.float32)
    xpp[:, 1:H + 5] = xp
    mpp[:, 1:H + 5] = mp
    maps = []
    for c in range(NCORES):
        r0 = c * ROWS
        xm = np.zeros((B, BATCH_ELEMS), np.float32)
        for b in range(B):
            for plane in range(2):
                P6 = xpp if plane == 0 else mpp
                for dh in (-1, 0, 1):
                    k = plane * 3 + (dh + 1)
                    # Q[k][sr, pc] = P[plane, sr+dh, pc]; P row sr lives at
                    # xpp row sr+1 -> rows (r0+dh+1) .. +SLAB
                    xm[b, 2 + k * PLANE:2 + (k + 1) * PLANE] = (
                        P6[b, r0 + dh + 1:r0 + dh + 1 + SLAB].ravel())
            xm[b, 2 + 6 * PLANE:2 + 7 * PLANE] = 1.0
        maps.append(xm.ravel().astype(BF16))
    return maps


def kernel(x, bn_gamma, bn_beta, bn_mean, bn_var, w1, b1, w2, b2):
    from concourse.bass_utils import run_bass_kernel_spmd

    if "nc" not in _cached:
        _cached["nc"] = _build_nc()
    nc = _cached["nc"]
    wts, bias1 = _prep_consts(bn_gamma, bn_beta, bn_mean, bn_var, w1, b1, w2, b2)
    xms = _prep_xm(x)
    in_maps = [{"xm": xms[c], "wts": wts, "bias1": bias1} for c in range(NCORES)]
    res = run_bass_kernel_spmd(nc, in_maps, list(range(NCORES)))
    full = np.empty((B, 64, H, W), np.float32)
    for c in range(NCORES):
        full[:, :, c * ROWS:(c + 1) * ROWS, :] = (
            np.asarray(res.results[c]["out"], np.float32).reshape(B, 64, ROWS, W))
    return full

